# revision 16
# baseline (speedup 1.0000x reference)
"""AFT-Local (sparse attention) Trainium2 kernel, 8-core data-parallel.

Problem: B=16, T=2048, D=256, window=128.
  q,k,v = x@Wq, x@Wk, x@Wv  (per batch)
  expw[t,s] = exp(where(|t-s|<128, w_bias, 0) - rowmax)   (stabilized)
  expk = exp(k - colmax)
  out = sigmoid(q) * (expw@(expk*v)) / (expw@expk) @ Wo

Math transform (exact in the num/den ratio -- the row/col shifts cancel):
  drop both stabilizations; expw = 1 + Wb with Wb = (exp(w_bias)-1)*band.
  num = S_kv + Wb_band @ (exp(k)*v),  den = S_k + Wb_band @ exp(k)
  where S_* are full-T column sums.  The dense [T,T] matmul becomes a
  width-255 banded matmul plus a rank-1 term (4.1x fewer flops).

Sharding: pure data-parallel over batch, 2 batches per core, weights and
w_bias replicated, no collectives.

Layout strategy (per batch, per core):
  - x cast to bf16 (chunked so compute starts early), xT [d,t] via
    TensorE transposes.
  - k,v fused: one matmul per (s-tile, d-chunk) against [Wk|Wv] (N=512);
    ek=exp(k), ekv=ek*v stored interleaved [ek_i|ekv_i] per tile so the
    S-sum reduction is 16 N=512 matmuls against a ones column.
  - qT in [d,t] (lhsT = Wq chunks, rhs = xT); sigmoid via the tanh
    identity (tanh shares the exp ACT table set; no table switches):
    sig = 0.5*(1+tanh(q/2)), the 0.5 folded into Wo.
  - band matmul: lhsT = ek/ekv [s,d] chunks, rhs = transposed-bias
    strips WbT[j] [s, t-window] -> num/den in [d,t]; the rank-1 S term
    is injected as a K=1 matmul into the same psum group.
  - rden = reciprocal_approx_fast(den_psum) on DVE (fp32, no ACT table
    switch; den is a sum of positive exps so well-conditioned).
  - y = (1+tanh(q/2)) * num * rden in [d,t] == yT, which is exactly the
    lhsT the output projection needs: out[t,e] = yT.T @ Wo. No y
    transposes.
  - all matmul operands bf16 (1 PE cycle/column + fast weight load; all
    accumulation is fp32 in PSUM -> end-to-end rel err ~4e-3).

Scheduling: w_bias strip DMAs (via gpsimd SWDGE, off the sync trigger
queue) + exp + mask run before batch 0 so ACT/DVE/DMA fill the head
while x loads; batch-0 projections give the PE dense work before the
strip transposes; long-lived per-batch tiles are double-buffered so
batch 1 overlaps batch 0's band phase.
"""

import numpy as np

B, T, D = 16, 2048, 256
WINDOW = 128
N_CORES = 8
B_LOC = B // N_CORES  # 2 batches per core
NT = T // 128  # 16 t/s tiles
NW = T // 256  # 8 band windows of 256 cols


def _build():
    import ml_dtypes
    import concourse.bacc as bacc
    import concourse.mybir as mybir
    import concourse.tile as tile

    f32 = mybir.dt.float32
    bf16 = mybir.dt.bfloat16
    AF = mybir.ActivationFunctionType
    OP = mybir.AluOpType

    nc = bacc.Bacc("TRN2", target_bir_lowering=False, debug=False,
                   num_devices=N_CORES)

    x_ext = nc.declare_dram_parameter("x", [B_LOC, T, D], f32, isOutput=False)
    wq_ext = nc.declare_dram_parameter("Wq", [D, D], f32, isOutput=False)
    wk_ext = nc.declare_dram_parameter("Wk", [D, D], f32, isOutput=False)
    wv_ext = nc.declare_dram_parameter("Wv", [D, D], f32, isOutput=False)
    wo_ext = nc.declare_dram_parameter("Wo", [D, D], f32, isOutput=False)
    wb_ext = nc.declare_dram_parameter("w_bias", [T, T], f32, isOutput=False)
    out_ext = nc.declare_dram_parameter("out", [B_LOC, T, D], f32, isOutput=True)

    # constants embedded in the NEFF (bf16)
    ident_np = np.eye(128, dtype=ml_dtypes.bfloat16)
    # band mask for a [128,384] strip of 3 stacked [t,s] blocks of s-tile j:
    # chunk0 = t-tile j-1 (band iff p>c), chunk1 = diag (all in band),
    # chunk2 = t-tile j+1 (band iff c>p)
    mU = np.tri(128, 128, -1, dtype=np.float32)
    mask_np = np.concatenate(
        [mU, np.ones((128, 128), np.float32), mU.T], axis=1
    ).astype(ml_dtypes.bfloat16)
    ident_dram = nc.inline_tensor(ident_np, name="ident")
    mask_dram = nc.inline_tensor(mask_np, name="bandmask")
    ones_dram = nc.inline_tensor(np.ones((128, 256), ml_dtypes.bfloat16),
                                 name="onesc")
    zeros_dram = nc.inline_tensor(np.zeros((128, 256), ml_dtypes.bfloat16),
                                  name="zeroc")

    def mm(out, lhsT, rhs, start, stop):
        nc.tensor.matmul(out, lhsT, rhs, start=start, stop=stop)

    with tile.TileContext(nc) as tc:
        with (
            tc.tile_pool(name="const", bufs=1) as cpool,
            tc.tile_pool(name="wbt", bufs=1) as wbtpool,
            tc.tile_pool(name="big", bufs=1) as bigpool,
            tc.tile_pool(name="db", bufs=2) as dbpool,
            tc.tile_pool(name="roll", bufs=4) as rollpool,
            tc.tile_pool(name="ps", bufs=8, space="PSUM") as pspool,
        ):
            # ---- constants (sync queue) ----
            ident = cpool.tile([128, 128], bf16, tag="ident")
            nc.sync.dma_start(ident[:], ident_dram[:])
            # ---- weights (gpsimd SWDGE, off the sync trigger queue) ----
            # wkv[dc] = [Wk | Wv] fused (one N=512 matmul per s-tile/d-chunk)
            wtmps = {}
            for name, ext in (("Wq", wq_ext), ("Wk", wk_ext),
                              ("Wv", wv_ext), ("Wo", wo_ext)):
                wtmp = rollpool.tile([128, 512], f32, tag="wtmp",
                                     name=f"wtmp{name}", bufs=4)
                nc.gpsimd.dma_start(
                    wtmp[:].rearrange("p (k c) -> p k c", c=256),
                    ext[:].rearrange("(k p) c -> p k c", p=128))
                wtmps[name] = wtmp
            wq, wo, wkv = {}, {}, {}
            for dc in range(2):
                wq[dc] = cpool.tile([128, 256], bf16, tag=f"wq{dc}",
                                    name=f"wq{dc}")
                nc.vector.tensor_copy(wq[dc][:],
                                      wtmps["Wq"][:, 256 * dc:256 * (dc + 1)])
                wo[dc] = cpool.tile([128, 256], bf16, tag=f"wo{dc}",
                                    name=f"wo{dc}")
                # fold the 0.5 of the sigmoid-tanh identity into Wo
                nc.vector.tensor_scalar_mul(
                    wo[dc][:], wtmps["Wo"][:, 256 * dc:256 * (dc + 1)], 0.5)
                wkv[dc] = cpool.tile([128, 512], bf16, tag=f"wkv{dc}",
                                     name=f"wkv{dc}")
                nc.vector.tensor_copy(wkv[dc][:, 0:256],
                                      wtmps["Wk"][:, 256 * dc:256 * (dc + 1)])
                nc.vector.tensor_copy(wkv[dc][:, 256:512],
                                      wtmps["Wv"][:, 256 * dc:256 * (dc + 1)])

            mask = cpool.tile([128, 384], bf16, tag="mask")
            nc.sync.dma_start(mask[:], mask_dram[:])
            ones_col = cpool.tile([128, 1], bf16, tag="ones_col")
            nc.sync.dma_start(ones_col[:], ones_dram[:, 0:1])
            ones_row = cpool.tile([1, 256], bf16, tag="ones_row")
            nc.sync.dma_start(ones_row[:], ones_dram[0:1, :])

            # ---- w_bias strips: DMA (gpsimd SWDGE) + exp + mask, early ----
            es_tiles = []
            strip_geo = []
            for j in range(NT):
                k_lo = 0 if j > 0 else 1       # which of the 3 blocks exist
                k_hi = 3 if j < NT - 1 else 2
                strip_geo.append((k_lo, k_hi))
                strip = rollpool.tile([128, 384], f32, tag="strip",
                                      name="strip")
                rows0 = 128 * (j - 1 + k_lo)
                nrows = 128 * (k_hi - k_lo)
                src = wb_ext[rows0:rows0 + nrows, 128 * j:128 * (j + 1)]
                nc.gpsimd.dma_start(
                    strip[:, 128 * k_lo:128 * k_hi].rearrange(
                        "p (k c) -> p k c", c=128),
                    src.rearrange("(k p) c -> p k c", p=128),
                )
                es = wbtpool.tile([128, 384], bf16, tag=f"es{j}",
                                  name=f"es{j}")
                es_tiles.append(es)
                sl = slice(128 * k_lo, 128 * k_hi)
                nc.scalar.activation(es[:, sl], strip[:, sl], AF.Exp)
                # (exp(w) - 1) * band, fused into one DVE op
                nc.vector.scalar_tensor_tensor(
                    es[:, sl], es[:, sl], -1.0, mask[:, sl],
                    op0=OP.add, op1=OP.mult)

            wbt = [wbtpool.tile([128, 640], bf16, tag=f"wbt{j}",
                                name=f"wbt{j}") for j in range(NT)]

            def build_wbt():
                # wbt[j] covers t in [128(j-2), 128(j+3)); cols [128,512)
                # hold the transposed band blocks, rest zeros.
                for j in range(NT):
                    wt, (k_lo, k_hi) = wbt[j], strip_geo[j]
                    lpad = 128 + 128 * k_lo
                    nc.gpsimd.dma_start(wt[:, 0:lpad], zeros_dram[:, 0:lpad])
                    if 128 + 128 * k_hi < 640:
                        rpad = 640 - (128 + 128 * k_hi)
                        nc.gpsimd.dma_start(wt[:, 128 + 128 * k_hi:640],
                                            zeros_dram[:, 0:rpad])
                    es = es_tiles[j]
                    tp = pspool.tile([128, 512], bf16, tag="ps", name="tp")
                    for k in range(k_lo, k_hi):
                        nc.tensor.transpose(tp[:, 128 * k:128 * (k + 1)],
                                            es[:, 128 * k:128 * (k + 1)],
                                            ident[:])
                    nc.vector.tensor_copy(
                        wt[:, 128 + 128 * k_lo:128 + 128 * k_hi],
                        tp[:, 128 * k_lo:128 * k_hi])

            def phase_a(b):
                """x load/cast/transpose + k,v,q projections for batch b."""
                xin = bigpool.tile([128, NT * 256], bf16, tag="xin",
                                   name="xin")
                for c in range(4):  # chunked load+cast so compute starts early
                    xf = rollpool.tile([128, 1024], f32, tag="xf", name="xf",
                                       bufs=3)
                    nsub = 4 if c == 0 else 2
                    for h in range(nsub):
                        w_t = 512 // nsub
                        t00 = 512 * c + w_t * h
                        nc.sync.dma_start(
                            xf[:, 2 * w_t * h:2 * w_t * (h + 1)].rearrange(
                                "p (n d) -> p n d", d=D),
                            x_ext[b, t00:t00 + w_t, :].rearrange(
                                "(n p) d -> p n d", p=128))
                    nc.vector.tensor_copy(
                        xin[:, 1024 * c:1024 * (c + 1)], xf[:])

                # xT chunks [d,t] (bf16)
                xT = [dbpool.tile([128, T], bf16, tag=f"xT{dc}",
                                  name=f"xT{dc}") for dc in range(2)]
                for r in range(4):  # 4 t-tiles per psum bank
                    for dc in range(2):
                        tp = pspool.tile([128, 512], bf16, tag="ps",
                                         name="tps")
                        for g in range(4):
                            i = 4 * r + g
                            nc.tensor.transpose(
                                tp[:, 128 * g:128 * (g + 1)],
                                xin[:, 256 * i + 128 * dc:
                                    256 * i + 128 * (dc + 1)],
                                ident[:])
                        nc.vector.tensor_copy(xT[dc][:, 512 * r:512 * (r + 1)],
                                              tp[:])

                # k|v fused projection; ekk holds [ek_i | ekv_i] interleaved
                ekk = dbpool.tile([128, NT * 512], bf16, tag="ekk",
                                  name="ekk")
                for i in range(NT):
                    kvp = pspool.tile([128, 512], f32, tag="ps", name="kvp")
                    for dc in range(2):
                        mm(kvp[:], xT[dc][:, 128 * i:128 * (i + 1)],
                           wkv[dc][:], dc == 0, dc == 1)
                    eksl = slice(512 * i, 512 * i + 256)
                    ekvsl = slice(512 * i + 256, 512 * (i + 1))
                    nc.scalar.activation(ekk[:, eksl], kvp[:, 0:256], AF.Exp)
                    nc.vector.tensor_mul(ekk[:, ekvsl], kvp[:, 256:512],
                                         ekk[:, eksl])

                # qT in [d,t] + tanh(q/2)  (bf16)
                tq = [dbpool.tile([128, T], bf16, tag=f"tq{dc}",
                                  name=f"tq{dc}") for dc in range(2)]
                for ec in range(2):
                    for r in range(4):
                        qp = pspool.tile([128, 512], f32, tag="ps", name="qp")
                        for dc in range(2):
                            mm(qp[:], wq[dc][:, 128 * ec:128 * (ec + 1)],
                               xT[dc][:, 512 * r:512 * (r + 1)],
                               dc == 0, dc == 1)
                        nc.scalar.activation(tq[ec][:, 512 * r:512 * (r + 1)],
                                             qp[:], AF.Tanh, scale=0.5)
                return ekk, tq

            def phase_b(b, ekk, tq):
                """S sums, band matmuls, epilogue, output projection."""
                # S: rhs = [ek_i | ekv_i] N=512 -> psum [1,512] = [S_k|S_kv]
                sp = pspool.tile([1, 512], f32, tag="ps", name="sp")
                for i in range(NT):
                    mm(sp[0:1, :], ones_col[:],
                       ekk[:, 512 * i:512 * (i + 1)], i == 0, i == NT - 1)
                s_sb = rollpool.tile([1, 512], bf16, tag="s_sb", name="s_sb",
                                     bufs=2)
                nc.scalar.activation(s_sb[:], sp[:], AF.Copy)

                y = [dbpool.tile([128, T], bf16, tag=f"y{dc}",
                                 name=f"y{dc}") for dc in range(2)]
                for dc in range(2):
                    for w in range(NW):
                        t0 = 256 * w
                        js = [j for j in range(2 * w - 1, 2 * w + 3)
                              if 0 <= j < NT]
                        nps = pspool.tile([128, 256], f32, tag="ps",
                                          name="nps")
                        dps = pspool.tile([128, 256], f32, tag="ps",
                                          name="dps")
                        for n, j in enumerate(js):
                            c0 = t0 - 128 * (j - 2)
                            rhs = wbt[j][:, c0:c0 + 256]
                            mm(nps[:],
                               ekk[:, 512 * j + 256 + 128 * dc:
                                   512 * j + 256 + 128 * (dc + 1)],
                               rhs, n == 0, False)
                            mm(dps[:],
                               ekk[:, 512 * j + 128 * dc:
                                   512 * j + 128 * (dc + 1)],
                               rhs, n == 0, False)
                        # rank-1 S injection closes each psum group
                        mm(nps[:], s_sb[0:1, 256 + 128 * dc:384 + 128 * dc],
                           ones_row[:], False, True)
                        mm(dps[:], s_sb[0:1, 128 * dc:128 * (dc + 1)],
                           ones_row[:], False, True)

                        rden = rollpool.tile([128, 256], f32, tag="rden",
                                             name="rden")
                        nc.vector.reciprocal_approx_fast(rden[:], dps[:])
                        ysl = y[dc][:, t0:t0 + 256]
                        nc.vector.tensor_mul(ysl, nps[:], rden[:])
                        # y *= (1 + tanh(q/2))   (0.5 folded into Wo)
                        nc.vector.scalar_tensor_tensor(
                            ysl, tq[dc][:, t0:t0 + 256], 1.0, ysl,
                            op0=OP.add, op1=OP.mult)

                # output projection out[t,e] = yT.T @ (0.5*Wo), evicted and
                # DMA'd per 2 t-tiles to keep the tail short
                for i in range(NT):
                    och = rollpool.tile([128, 256], f32, tag="och",
                                        name="och", bufs=4)
                    op = pspool.tile([128, 256], f32, tag="ps", name="op")
                    for dc in range(2):
                        mm(op[:], y[dc][:, 128 * i:128 * (i + 1)],
                           wo[dc][:], dc == 0, dc == 1)
                    nc.scalar.activation(och[:], op[:], AF.Copy)
                    nc.gpsimd.dma_start(out_ext[b, 128 * i:128 * (i + 1), :],
                                        och[:])

            # issue order: batch-0 projections before the strip transposes
            # (dense PE work first), then band; batch 1 overlaps via
            # double-buffered tiles.
            ctx0 = phase_a(0)
            build_wbt()
            phase_b(0, *ctx0)
            ctx1 = phase_a(1)
            phase_b(1, *ctx1)

    nc.compile()
    return nc


_NC_CACHE = None


def kernel(x, Wq, Wk, Wv, Wo, w_bias, window=None):
    from concourse.bass_utils import run_bass_kernel_spmd

    global _NC_CACHE
    if _NC_CACHE is None:
        _NC_CACHE = _build()
    nc = _NC_CACHE

    x = np.ascontiguousarray(np.asarray(x, dtype=np.float32))
    w_bias = np.ascontiguousarray(np.asarray(w_bias, dtype=np.float32))
    wmats = [np.ascontiguousarray(np.asarray(w, dtype=np.float32))
             for w in (Wq, Wk, Wv, Wo)]

    in_maps = []
    for c in range(N_CORES):
        in_maps.append({
            "x": x[B_LOC * c:B_LOC * (c + 1)],
            "Wq": wmats[0], "Wk": wmats[1], "Wv": wmats[2], "Wo": wmats[3],
            "w_bias": w_bias,
        })
    res = run_bass_kernel_spmd(nc, in_maps, core_ids=list(range(N_CORES)))
    return np.concatenate([res.results[c]["out"] for c in range(N_CORES)],
                          axis=0)


# revision 17
# speedup vs baseline: 1.0747x; 1.0747x over previous
"""AFT-Local (sparse attention) Trainium2 kernel, 8-core data-parallel.

Problem: B=16, T=2048, D=256, window=128.
  q,k,v = x@Wq, x@Wk, x@Wv  (per batch)
  expw[t,s] = exp(where(|t-s|<128, w_bias, 0) - rowmax)   (stabilized)
  expk = exp(k - colmax)
  out = sigmoid(q) * (expw@(expk*v)) / (expw@expk) @ Wo

Math transform (exact in the num/den ratio -- the row/col shifts cancel):
  drop both stabilizations; expw = 1 + Wb with Wb = (exp(w_bias)-1)*band.
  num = S_kv + Wb_band @ (exp(k)*v),  den = S_k + Wb_band @ exp(k)
  where S_* are full-T column sums.  The dense [T,T] matmul becomes a
  width-255 banded matmul plus a rank-1 term (4.1x fewer flops).

Sharding: pure data-parallel over batch, 2 batches per core, weights and
w_bias replicated, no collectives.

Layout strategy (per batch, per core):
  - x cast to bf16 (chunked so compute starts early), xT [d,t] via
    TensorE transposes.
  - k,v fused: one matmul per (s-tile, d-chunk) against [Wk|Wv] (N=512);
    ek=exp(k), ekv=ek*v stored interleaved [ek_i|ekv_i] per tile so the
    S-sum reduction is 16 N=512 matmuls against a ones column.
  - qT in [d,t] (lhsT = Wq chunks, rhs = xT); sigmoid via the tanh
    identity (tanh shares the exp ACT table set; no table switches):
    sig = 0.5*(1+tanh(q/2)), the 0.5 folded into Wo.
  - band matmul: lhsT = ek/ekv [s,d] chunks, rhs = transposed-bias
    strips WbT[j] [s, t-window] -> num/den in [d,t]; the rank-1 S term
    is injected as a K=1 matmul into the same psum group.
  - rden = reciprocal_approx_fast(den_psum) on DVE (fp32, no ACT table
    switch; den is a sum of positive exps so well-conditioned).
  - y = (1+tanh(q/2)) * num * rden in [d,t] == yT, which is exactly the
    lhsT the output projection needs: out[t,e] = yT.T @ Wo. No y
    transposes.
  - all matmul operands bf16 (1 PE cycle/column + fast weight load; all
    accumulation is fp32 in PSUM -> end-to-end rel err ~4e-3).

Scheduling: w_bias strip DMAs (via gpsimd SWDGE, off the sync trigger
queue) + exp + mask run before batch 0 so ACT/DVE/DMA fill the head
while x loads; batch-0 projections give the PE dense work before the
strip transposes; long-lived per-batch tiles are double-buffered so
batch 1 overlaps batch 0's band phase.
"""

import numpy as np

B, T, D = 16, 2048, 256
WINDOW = 128
N_CORES = 8
B_LOC = B // N_CORES  # 2 batches per core
NT = T // 128  # 16 t/s tiles
NW = T // 256  # 8 band windows of 256 cols


def _build():
    import ml_dtypes
    import concourse.bacc as bacc
    import concourse.mybir as mybir
    import concourse.tile as tile

    f32 = mybir.dt.float32
    bf16 = mybir.dt.bfloat16
    AF = mybir.ActivationFunctionType
    OP = mybir.AluOpType

    nc = bacc.Bacc("TRN2", target_bir_lowering=False, debug=False,
                   num_devices=N_CORES)

    x_ext = nc.declare_dram_parameter("x", [B_LOC, T, D], f32, isOutput=False)
    wq_ext = nc.declare_dram_parameter("Wq", [D, D], f32, isOutput=False)
    wk_ext = nc.declare_dram_parameter("Wk", [D, D], f32, isOutput=False)
    wv_ext = nc.declare_dram_parameter("Wv", [D, D], f32, isOutput=False)
    wo_ext = nc.declare_dram_parameter("Wo", [D, D], f32, isOutput=False)
    wb_ext = nc.declare_dram_parameter("w_bias", [T, T], f32, isOutput=False)
    out_ext = nc.declare_dram_parameter("out", [B_LOC, T, D], f32, isOutput=True)

    # constants embedded in the NEFF (bf16)
    ident_np = np.eye(128, dtype=ml_dtypes.bfloat16)
    # band mask for a [128,384] strip of 3 stacked [t,s] blocks of s-tile j:
    # chunk0 = t-tile j-1 (band iff p>c), chunk1 = diag (all in band),
    # chunk2 = t-tile j+1 (band iff c>p)
    mU = np.tri(128, 128, -1, dtype=np.float32)
    mask_np = np.concatenate(
        [mU, np.ones((128, 128), np.float32), mU.T], axis=1
    ).astype(ml_dtypes.bfloat16)
    ident_dram = nc.inline_tensor(ident_np, name="ident")
    mask_dram = nc.inline_tensor(mask_np, name="bandmask")
    ones_dram = nc.inline_tensor(np.ones((128, 256), ml_dtypes.bfloat16),
                                 name="onesc")
    zeros_dram = nc.inline_tensor(np.zeros((128, 256), ml_dtypes.bfloat16),
                                  name="zeroc")

    def mm(out, lhsT, rhs, start, stop):
        nc.tensor.matmul(out, lhsT, rhs, start=start, stop=stop)

    with tile.TileContext(nc) as tc:
        with (
            tc.tile_pool(name="const", bufs=1) as cpool,
            tc.tile_pool(name="wbt", bufs=1) as wbtpool,
            tc.tile_pool(name="big", bufs=1) as bigpool,
            tc.tile_pool(name="db", bufs=2) as dbpool,
            tc.tile_pool(name="roll", bufs=4) as rollpool,
            tc.tile_pool(name="ps", bufs=8, space="PSUM") as pspool,
        ):
            # ---- constants (sync queue) ----
            ident = cpool.tile([128, 128], bf16, tag="ident")
            nc.sync.dma_start(ident[:], ident_dram[:])
            mask = cpool.tile([128, 384], bf16, tag="mask")
            nc.sync.dma_start(mask[:], mask_dram[:])
            ones_col = cpool.tile([128, 1], bf16, tag="ones_col")
            nc.sync.dma_start(ones_col[:], ones_dram[:, 0:1])
            ones_row = cpool.tile([1, 256], bf16, tag="ones_row")
            nc.sync.dma_start(ones_row[:], ones_dram[0:1, :])

            # ---- weights (gpsimd SWDGE, off the sync trigger queue) ----
            # wkv[dc] = [Wk | Wv] fused (one N=512 matmul per s-tile/d-chunk)
            wtmps = {}
            for name, ext in (("Wq", wq_ext), ("Wk", wk_ext),
                              ("Wv", wv_ext), ("Wo", wo_ext)):
                wtmp = rollpool.tile([128, 512], f32, tag="wtmp",
                                     name=f"wtmp{name}", bufs=4)
                nc.gpsimd.dma_start(
                    wtmp[:].rearrange("p (k c) -> p k c", c=256),
                    ext[:].rearrange("(k p) c -> p k c", p=128))
                wtmps[name] = wtmp
            wq, wo, wkv = {}, {}, {}
            for dc in range(2):
                wq[dc] = cpool.tile([128, 256], bf16, tag=f"wq{dc}",
                                    name=f"wq{dc}")
                nc.vector.tensor_copy(wq[dc][:],
                                      wtmps["Wq"][:, 256 * dc:256 * (dc + 1)])
                wo[dc] = cpool.tile([128, 256], bf16, tag=f"wo{dc}",
                                    name=f"wo{dc}")
                # fold the 0.5 of the sigmoid-tanh identity into Wo
                nc.vector.tensor_scalar_mul(
                    wo[dc][:], wtmps["Wo"][:, 256 * dc:256 * (dc + 1)], 0.5)
                wkv[dc] = cpool.tile([128, 512], bf16, tag=f"wkv{dc}",
                                     name=f"wkv{dc}")
                nc.vector.tensor_copy(wkv[dc][:, 0:256],
                                      wtmps["Wk"][:, 256 * dc:256 * (dc + 1)])
                nc.vector.tensor_copy(wkv[dc][:, 256:512],
                                      wtmps["Wv"][:, 256 * dc:256 * (dc + 1)])

            # ---- w_bias strips: DMA (gpsimd SWDGE) + exp + mask, early ----
            es_tiles = []
            strip_geo = []
            for j in range(NT):
                k_lo = 0 if j > 0 else 1       # which of the 3 blocks exist
                k_hi = 3 if j < NT - 1 else 2
                strip_geo.append((k_lo, k_hi))
                strip = rollpool.tile([128, 384], f32, tag="strip",
                                      name="strip")
                rows0 = 128 * (j - 1 + k_lo)
                nrows = 128 * (k_hi - k_lo)
                src = wb_ext[rows0:rows0 + nrows, 128 * j:128 * (j + 1)]
                nc.gpsimd.dma_start(
                    strip[:, 128 * k_lo:128 * k_hi].rearrange(
                        "p (k c) -> p k c", c=128),
                    src.rearrange("(k p) c -> p k c", p=128),
                )
                es = wbtpool.tile([128, 384], bf16, tag=f"es{j}",
                                  name=f"es{j}")
                es_tiles.append(es)
                sl = slice(128 * k_lo, 128 * k_hi)
                nc.scalar.activation(es[:, sl], strip[:, sl], AF.Exp)
                # (exp(w) - 1) * band, fused into one DVE op
                nc.vector.scalar_tensor_tensor(
                    es[:, sl], es[:, sl], -1.0, mask[:, sl],
                    op0=OP.add, op1=OP.mult)

            wbt = [wbtpool.tile([128, 640], bf16, tag=f"wbt{j}",
                                name=f"wbt{j}") for j in range(NT)]

            def build_wbt():
                # wbt[j] covers t in [128(j-2), 128(j+3)); cols [128,512)
                # hold the transposed band blocks, rest zeros.
                for j in range(NT):
                    wt, (k_lo, k_hi) = wbt[j], strip_geo[j]
                    lpad = 128 + 128 * k_lo
                    nc.gpsimd.dma_start(wt[:, 0:lpad], zeros_dram[:, 0:lpad])
                    if 128 + 128 * k_hi < 640:
                        rpad = 640 - (128 + 128 * k_hi)
                        nc.gpsimd.dma_start(wt[:, 128 + 128 * k_hi:640],
                                            zeros_dram[:, 0:rpad])
                    es = es_tiles[j]
                    tp = pspool.tile([128, 512], bf16, tag="ps", name="tp")
                    for k in range(k_lo, k_hi):
                        nc.tensor.transpose(tp[:, 128 * k:128 * (k + 1)],
                                            es[:, 128 * k:128 * (k + 1)],
                                            ident[:])
                    nc.vector.tensor_copy(
                        wt[:, 128 + 128 * k_lo:128 + 128 * k_hi],
                        tp[:, 128 * k_lo:128 * k_hi])

            def phase_a(b):
                """x load/cast/transpose + k,v,q projections for batch b."""
                xin = bigpool.tile([128, NT * 256], bf16, tag="xin",
                                   name="xin")
                for c in range(4):  # chunked load+cast so compute starts early
                    xf = rollpool.tile([128, 1024], f32, tag="xf", name="xf",
                                       bufs=3)
                    nsub = 2
                    for h in range(nsub):
                        w_t = 512 // nsub
                        t00 = 512 * c + w_t * h
                        nc.sync.dma_start(
                            xf[:, 2 * w_t * h:2 * w_t * (h + 1)].rearrange(
                                "p (n d) -> p n d", d=D),
                            x_ext[b, t00:t00 + w_t, :].rearrange(
                                "(n p) d -> p n d", p=128))
                    nc.vector.tensor_copy(
                        xin[:, 1024 * c:1024 * (c + 1)], xf[:])

                # xT chunks [d,t] (bf16)
                xT = [dbpool.tile([128, T], bf16, tag=f"xT{dc}",
                                  name=f"xT{dc}") for dc in range(2)]
                for r in range(4):  # 4 t-tiles per psum bank
                    for dc in range(2):
                        tp = pspool.tile([128, 512], bf16, tag="ps",
                                         name="tps")
                        for g in range(4):
                            i = 4 * r + g
                            nc.tensor.transpose(
                                tp[:, 128 * g:128 * (g + 1)],
                                xin[:, 256 * i + 128 * dc:
                                    256 * i + 128 * (dc + 1)],
                                ident[:])
                        nc.vector.tensor_copy(xT[dc][:, 512 * r:512 * (r + 1)],
                                              tp[:])

                # k|v fused projection; ekk holds [ek_i | ekv_i] interleaved
                ekk = dbpool.tile([128, NT * 512], bf16, tag="ekk",
                                  name="ekk")
                for i in range(NT):
                    kvp = pspool.tile([128, 512], f32, tag="ps", name="kvp")
                    for dc in range(2):
                        mm(kvp[:], xT[dc][:, 128 * i:128 * (i + 1)],
                           wkv[dc][:], dc == 0, dc == 1)
                    eksl = slice(512 * i, 512 * i + 256)
                    ekvsl = slice(512 * i + 256, 512 * (i + 1))
                    nc.scalar.activation(ekk[:, eksl], kvp[:, 0:256], AF.Exp)
                    nc.vector.tensor_mul(ekk[:, ekvsl], kvp[:, 256:512],
                                         ekk[:, eksl])

                # qT in [d,t] + tanh(q/2)  (bf16)
                tq = [dbpool.tile([128, T], bf16, tag=f"tq{dc}",
                                  name=f"tq{dc}") for dc in range(2)]
                for ec in range(2):
                    for r in range(4):
                        qp = pspool.tile([128, 512], f32, tag="ps", name="qp")
                        for dc in range(2):
                            mm(qp[:], wq[dc][:, 128 * ec:128 * (ec + 1)],
                               xT[dc][:, 512 * r:512 * (r + 1)],
                               dc == 0, dc == 1)
                        nc.scalar.activation(tq[ec][:, 512 * r:512 * (r + 1)],
                                             qp[:], AF.Tanh, scale=0.5)
                return ekk, tq

            def phase_b(b, ekk, tq):
                """S sums, band matmuls, epilogue, output projection."""
                # S: rhs = [ek_i | ekv_i] N=512 -> psum [1,512] = [S_k|S_kv]
                sp = pspool.tile([1, 512], f32, tag="ps", name="sp")
                for i in range(NT):
                    mm(sp[0:1, :], ones_col[:],
                       ekk[:, 512 * i:512 * (i + 1)], i == 0, i == NT - 1)
                s_sb = rollpool.tile([1, 512], bf16, tag="s_sb", name="s_sb",
                                     bufs=2)
                nc.scalar.activation(s_sb[:], sp[:], AF.Copy)

                y = [dbpool.tile([128, T], bf16, tag=f"y{dc}",
                                 name=f"y{dc}") for dc in range(2)]
                for dc in range(2):
                    for w in range(NW):
                        t0 = 256 * w
                        js = [j for j in range(2 * w - 1, 2 * w + 3)
                              if 0 <= j < NT]
                        nps = pspool.tile([128, 256], f32, tag="ps",
                                          name="nps")
                        dps = pspool.tile([128, 256], f32, tag="ps",
                                          name="dps")
                        for n, j in enumerate(js):
                            c0 = t0 - 128 * (j - 2)
                            rhs = wbt[j][:, c0:c0 + 256]
                            mm(nps[:],
                               ekk[:, 512 * j + 256 + 128 * dc:
                                   512 * j + 256 + 128 * (dc + 1)],
                               rhs, n == 0, False)
                            mm(dps[:],
                               ekk[:, 512 * j + 128 * dc:
                                   512 * j + 128 * (dc + 1)],
                               rhs, n == 0, False)
                        # rank-1 S injection closes each psum group
                        mm(nps[:], s_sb[0:1, 256 + 128 * dc:384 + 128 * dc],
                           ones_row[:], False, True)
                        mm(dps[:], s_sb[0:1, 128 * dc:128 * (dc + 1)],
                           ones_row[:], False, True)

                        rden = rollpool.tile([128, 256], f32, tag="rden",
                                             name="rden")
                        nc.vector.reciprocal_approx_fast(rden[:], dps[:])
                        ysl = y[dc][:, t0:t0 + 256]
                        nc.vector.tensor_mul(ysl, nps[:], rden[:])
                        # y *= (1 + tanh(q/2))   (0.5 folded into Wo)
                        nc.vector.scalar_tensor_tensor(
                            ysl, tq[dc][:, t0:t0 + 256], 1.0, ysl,
                            op0=OP.add, op1=OP.mult)

                # output projection out[t,e] = yT.T @ (0.5*Wo), evicted and
                # DMA'd per 2 t-tiles to keep the tail short
                for r in range(8):
                    och = rollpool.tile([128, 512], f32, tag="och",
                                        name="och", bufs=4)
                    for g in range(2):
                        i = 2 * r + g
                        op = pspool.tile([128, 256], f32, tag="ps", name="op")
                        for dc in range(2):
                            mm(op[:], y[dc][:, 128 * i:128 * (i + 1)],
                               wo[dc][:], dc == 0, dc == 1)
                        nc.scalar.activation(och[:, 256 * g:256 * (g + 1)],
                                             op[:], AF.Copy)
                    nc.sync.dma_start(
                        out_ext[b, 256 * r:256 * (r + 1), :].rearrange(
                            "(n p) d -> p n d", p=128),
                        och[:].rearrange("p (n d) -> p n d", d=D))

            # issue order: batch-0 projections before the strip transposes
            # (dense PE work first), then band; batch 1 overlaps via
            # double-buffered tiles.
            ctx0 = phase_a(0)
            build_wbt()
            phase_b(0, *ctx0)
            ctx1 = phase_a(1)
            phase_b(1, *ctx1)

    nc.compile()
    return nc


_NC_CACHE = None


def kernel(x, Wq, Wk, Wv, Wo, w_bias, window=None):
    from concourse.bass_utils import run_bass_kernel_spmd

    global _NC_CACHE
    if _NC_CACHE is None:
        _NC_CACHE = _build()
    nc = _NC_CACHE

    x = np.ascontiguousarray(np.asarray(x, dtype=np.float32))
    w_bias = np.ascontiguousarray(np.asarray(w_bias, dtype=np.float32))
    wmats = [np.ascontiguousarray(np.asarray(w, dtype=np.float32))
             for w in (Wq, Wk, Wv, Wo)]

    in_maps = []
    for c in range(N_CORES):
        in_maps.append({
            "x": x[B_LOC * c:B_LOC * (c + 1)],
            "Wq": wmats[0], "Wk": wmats[1], "Wv": wmats[2], "Wo": wmats[3],
            "w_bias": w_bias,
        })
    res = run_bass_kernel_spmd(nc, in_maps, core_ids=list(range(N_CORES)))
    return np.concatenate([res.results[c]["out"] for c in range(N_CORES)],
                          axis=0)
